# revision 3
# baseline (speedup 1.0000x reference)
"""Trainium2 Bass kernel for nn_BondLenConstrain (peptide-bond gaussian NLL).

Contract: kernel(**inputs) takes the FULL unsharded inputs (as produced by
reference.setup_inputs()) and returns the FULL [B, CH, R, NALT] output.

Strategy (v2)
-------------
Structured input layout (verified on host): atom index = ((b*CH+ch)*R + r)*3
+ at, every bond valid; mean/std rows identical -> the gaussian NLL folds to
per-feature clamped parabolas  score_f = min((a_f*x_f + b_f)^2, C_f).

Data-parallel over batch: core i handles batches [2i, 2i+2).  Per batch the
coords are loaded as overlapped 585-float partition rows (residues
[64p, 64p+64] inclusive), so residue r+1 is always in-partition and all four
bond atoms are strided views.  Per half-batch unit (4 chains):
  TMP = S[j+3]-S[j]            one contiguous subtract = all diff vectors
  SQ  = TMP^2                  (Act)
  NS  = windowed sum3(SQ)      all three norms, interleaved stride 3
  MUL = TMP[9k+3..9]*TMP[9k+6..12]   both dots' products
  DT  = sum3(MUL) planes [raw2 | d1]
  PC  = na2*nb planes;  QQ = PC - DT^2   (custom DVE op)
  RQ  = abs_rsqrt(QQ) (Act; arctan saturates so no clip needed)
  TC  = DT*RQ;  AR = arctan(TC) (Act, single table switch per core)
  ACC = min((na2*rna*a0+b0)^2,C0) then chained custom WMIN adds
Output: compact per-bond scores DMA'd out; the [B,CH,R,NALT] slab is
assembled host-side during unshard (only alt=0, residues <R-1 are nonzero).

Engine placement balances DVE / Pool / Act with measured rates; activation
tables: abs_rsqrt+square phase, then one switch to trig for arctan.
"""

import numpy as np

B, CH, R, NALT = 16, 8, 8192, 10
EPS = 1e-10
NCORES = 8
BPC = B // NCORES            # batches per core = 2
K = 64                       # residues per partition (128*64 = 8192 = R)
CW = 9 * K + 9               # loaded floats per chain-row = 585 (overlap 9)
TW = 582                     # TMP width per chain
HC = CH // 2                 # chains per half-unit
CHAIN_F = R * 9              # floats per chain = 73728
GRP_F = CH * CHAIN_F         # floats per batch = 589824
CORE_F = BPC * GRP_F         # coords floats per core = 1179648
DEG = 180.0 / np.pi
NB = HC * K                  # bonds per partition per half-unit = 256

_BUILT = {}


# ---------------------------------------------------------------- custom ops
def _register_dve_ops():
    import concourse.dve_ops as dvo
    from concourse.dve_spec import (
        Spec, Src0, Src1, C0, C1, C2, lower, maxx, minn, sq, _has_src1,
    )
    from concourse.dve_uop import DveOpSpec

    def mk(name, spec):
        for o in dvo.OPS:
            if o.name == name:
                return o
        row = dvo._CUSTOM_DVE_ROW_BASE + len(dvo.OPS)
        assert row < 0x20, "custom DVE op rows exhausted"
        shas = {}
        for ver in ("v3", "v4"):
            u = lower(spec, ver=ver)
            shas[ver] = DveOpSpec(
                name=name, opcode=row, uops=u, rd1_en=_has_src1(spec)
            ).sha(ver)
        op = dvo.DveOp(name, spec, subdim=False, uops_sha=shas)
        dvo.OPS.append(op)
        dvo.CUSTOM_DVE_SPECS[name] = spec
        dvo._SUB_OPCODE_FOR_NAME[name] = row
        return op

    ops = {}
    # q = max(pcat - (s1*dot)^2, eps)
    ops["QSUB"] = mk("ANT_BL_QSUB", Spec(
        body=maxx(Src0 - sq(Src1 * C1), C0),
        reference=lambda in0, in1, s0, s1, imm2:
            np.maximum(in0 - (in1.astype(np.float32) * s1) ** 2, s0
                       ).astype(np.float32),
    ))
    # acc0 = min((na2*rna*a + b)^2, C)   (blen = na2*rna)
    ops["WMIN0B"] = mk("ANT_BL_WMIN0B", Spec(
        body=minn(sq(Src0 * Src1 * C0 + C1), C2),
        reference=lambda in0, in1, s0, s1, imm2:
            np.minimum((in0 * in1 * s0 + s1).astype(np.float32) ** 2, imm2
                       ).astype(np.float32),
    ))
    # acc = min((x*a + b)^2, C) + acc_prev
    ops["WMIN"] = mk("ANT_BL_WMIN", Spec(
        body=minn(sq(Src0 * C0 + C1), C2) + Src1,
        reference=lambda in0, in1, s0, s1, imm2:
            (np.minimum((in0 * s0 + s1).astype(np.float32) ** 2, imm2) + in1
             ).astype(np.float32),
    ))
    return ops


# ------------------------------------------------------------- host helpers
def _check_structured(atom_description, coords, mean, std, weight):
    if atom_description.shape != (B * CH * R * 3, 5):
        return False
    if coords.shape != (B * CH * R * 3, 3):
        return False
    if mean.shape != (20, 3) or std.shape != (20, 3) or weight.shape != (1,):
        return False
    if not ((mean == mean[0]).all() and (std == std[0]).all()):
        return False
    ad = atom_description
    n = B * CH * R
    at = np.tile(np.array([0, 1, 2], dtype=ad.dtype), n)
    if not np.array_equal(ad[:, 0], at):
        return False
    r = np.repeat(np.tile(np.arange(R, dtype=ad.dtype), B * CH), 3)
    if not np.array_equal(ad[:, 1], r):
        return False
    c = np.repeat(np.tile(np.arange(CH, dtype=ad.dtype), B), R * 3)
    if not np.array_equal(ad[:, 2], c):
        return False
    b = np.repeat(np.arange(B, dtype=ad.dtype), CH * R * 3)
    if not np.array_equal(ad[:, 3], b):
        return False
    return True


def _consts(mean, std, weight):
    """Fold mean/std/weight into per-feature device constants."""
    mu = mean[0].astype(np.float64)        # [3]
    var = std[0].astype(np.float64) ** 2   # [3]
    denom = np.sqrt(2.0 * np.pi * var)
    scale = float(1.0 - np.tanh(-np.float64(weight[0])))
    hiv = scale / (2.0 * var)
    Cs = (-np.log(EPS) - np.log(denom)) * scale
    a0 = np.sqrt(hiv[0])
    b0 = -mu[0] * a0
    a1 = -DEG * np.sqrt(hiv[1])
    b1 = (DEG * np.pi / 2.0 - mu[1]) * np.sqrt(hiv[1])
    a2 = -DEG * np.sqrt(hiv[2])
    b2 = (DEG * np.pi / 2.0 - mu[2]) * np.sqrt(hiv[2])
    # sanity: the clamp band must sit inside (0, 180) so that the arctan
    # formulation (t = cot(ang), saturating table) covers it
    d1 = np.sqrt(Cs[1] / hiv[1])
    d2 = np.sqrt(Cs[2] / hiv[2])
    ang_lo = max(min(mu[1] - d1, mu[2] - d2), 0.0)
    ang_hi = min(max(mu[1] + d1, mu[2] + d2), 180.0)
    if not (0.0 < ang_lo and ang_hi < 180.0):
        return None
    vals = [a0, b0, Cs[0], a1, b1, Cs[1], a2, b2, Cs[2]]
    return tuple(np.float32(v) for v in vals)


# ------------------------------------------------------------------- device
# build knobs, tuned from HW probes
USE_F16 = True        # fp16 MUL/DT/PC/RQ intermediates
DT_VIA_TR = True      # DT via tensor_reduce (else strided adds)
SQ_SCALE = 0.125      # fold 1/64 into squared norms to keep pcat in range


def _build(consts):
    import concourse.bacc as bacc
    import concourse.bass as bass
    import concourse.mybir as mybir
    from concourse.alu_op_type import AluOpType as alu
    from concourse.tile import TileContext

    OPS = _register_dve_ops()
    a0, b0, C0, a1, b1, C1, a2, b2, C2 = (float(v) for v in consts)
    f32 = mybir.dt.float32
    f16 = mybir.dt.float16 if USE_F16 else mybir.dt.float32
    AF = mybir.ActivationFunctionType
    ssc = SQ_SCALE if USE_F16 else 1.0     # scale applied inside Act Square
    isc = 1.0 / ssc                        # na2 etc are scaled by ssc^2

    nc = bacc.Bacc("TRN2", target_bir_lowering=False, debug=False)
    # sarr = coords repacked host-side into the exact SBUF slab layout
    # [g][p][c][j(585)], so every load chunk is a contiguous partition row
    SPW = CH * CW  # per-partition floats per group = 4680
    sarr = nc.dram_tensor("sarr", [BPC * 128 * SPW], f32, kind="ExternalInput")
    out = nc.dram_tensor("out", [BPC * 2 * 128 * NB], f32, kind="ExternalOutput")

    GNB = CH * K  # bonds per partition per group = 512
    with TileContext(nc) as tc:
        with (
            tc.tile_pool(name="io", bufs=1) as io,
            tc.tile_pool(name="wk", bufs=1) as wk,
        ):
            # preload the abs_rsqrt(+square) activation table during DMA wait
            scr = wk.tile([128, 8], f32, tag="scr")
            nc.vector.memset(scr[:], 0.0)
            nc.scalar.activation(scr[:, 0:1], scr[:, 0:1], AF.Abs_reciprocal_sqrt)
            # per-group arctan gates (written after the group's abs_rsqrt)
            gates = {}
            for g in range(BPC):
                gate = wk.tile([128, 1], f32, tag=f"gate{g}")
                gates[g] = gate

            # chunk sizes in chains; fully contiguous DMA rows
            CHUNKS = [2, 2, 2, 2]
            units = []
            for g in range(BPC):
                S = io.tile([128, CH * CW], f32, tag=f"S{g}")
                c0 = 0
                for cn in CHUNKS:
                    nc.sync.dma_start(
                        S[:, c0 * CW:(c0 + cn) * CW],
                        bass.AP(sarr, g * 128 * SPW + c0 * CW,
                                [[SPW, 128], [1, cn * CW]]),
                    )
                    c0 += cn
                units.append((g, S))

            tcs, accs = {}, {}
            for g, S in units:
                TMP = wk.tile([128, CH * TW], f32, tag=f"TMP{g}")
                SQ = wk.tile([128, CH * TW], f32, tag=f"SQ{g}")
                NS = wk.tile([128, CH * 194], f32, tag=f"NS{g}")
                MUL = wk.tile([128, CH * K * 6], f16, tag=f"MUL{g}")
                DT = wk.tile([128, 2 * GNB], f16, tag=f"DT{g}")
                PC = wk.tile([128, 2 * GNB], f16, tag=f"PC{g}")
                QQ = wk.tile([128, 2 * GNB], f32, tag=f"QQ{g}")
                RQ = wk.tile([128, 2 * GNB], f16, tag=f"RQ{g}")
                RN = wk.tile([128, GNB], f32, tag=f"RN{g}")
                TC = wk.tile([128, 2 * GNB], f32, tag=f"TC{g}")
                ACC = wk.tile([128, GNB], f32, tag=f"ACC{g}")
                tcs[g] = TC
                accs[g] = ACC

                def sv(off, c0, cn):
                    return bass.AP(S.tensor, S.offset + c0 * CW + off,
                                   [S.ap[0], [CW, cn], [1, TW]])

                # TMP = S[j+3]-S[j]  (DVE; two 4-chain halves)
                tmp3 = TMP[:].rearrange("p (c j) -> p c j", c=CH)
                for q in range(2):
                    nc.vector.tensor_tensor(
                        tmp3[:, 4 * q:4 * q + 4],
                        sv(3, 4 * q, 4), sv(0, 4 * q, 4), alu.subtract)

                # SQ = (ssc*TMP)^2  (Act; square lives in every table)
                # split in chain-halves so downstream ops pipeline sooner
                HW_ = 4 * TW
                for q in range(2):
                    nc.scalar.activation(SQ[:, q * HW_:(q + 1) * HW_],
                                         TMP[:, q * HW_:(q + 1) * HW_],
                                         AF.Square, scale=ssc)

                # MUL = TMP[9k+3..9] * TMP[9k+6..12]  (DVE, fp16 out)
                def tv(off, c0, cn):
                    return bass.AP(TMP.tensor, TMP.offset + c0 * TW + off,
                                   [TMP.ap[0], [TW, cn], [9, K], [1, 6]])
                mul4 = MUL[:].rearrange("p (c k t) -> p c k t", c=CH, t=6)
                for q in range(2):
                    nc.vector.tensor_tensor(
                        mul4[:, 4 * q:4 * q + 4],
                        tv(3, 4 * q, 4), tv(6, 4 * q, 4), alu.mult)

                # NS = sum3(SQ)  (DVE adds, stride-3 reads)
                def sqv(off, c0, cn):
                    return bass.AP(SQ.tensor, SQ.offset + c0 * TW + off,
                                   [SQ.ap[0], [TW, cn], [3, 194]])
                ns3 = NS[:].rearrange("p (c m) -> p c m", c=CH)
                for q in range(2):
                    sl = ns3[:, 4 * q:4 * q + 4]
                    nc.vector.tensor_tensor(
                        sl, sqv(0, 4 * q, 4), sqv(1, 4 * q, 4), alu.add)
                    nc.vector.tensor_tensor(sl, sl, sqv(2, 4 * q, 4), alu.add)

                # DT planes [raw2 | d1] = sum3(MUL)  (DVE tensor_reduce)
                with nc.allow_low_precision(reason="dot sums fit fp16"):
                    for t in range(2):
                        pl = DT[:, t * GNB:(t + 1) * GNB].rearrange(
                            "p (c k) -> p c k", c=CH)
                        nc.vector.tensor_reduce(
                            pl,
                            bass.AP(MUL.tensor, MUL.offset + 3 * t,
                                    [MUL.ap[0], [K * 6, CH], [6, K], [1, 3]]),
                            mybir.AxisListType.X, alu.add)

                # PC planes: [na2*nb2 | na2*nb1]  (DVE, stride-3 reads)
                def nsv(off):
                    return bass.AP(NS.tensor, NS.offset + off,
                                   [NS.ap[0], [194, CH], [3, K]])
                for t, nb_off in ((0, 1), (1, 3)):
                    pl = PC[:, t * GNB:(t + 1) * GNB].rearrange(
                        "p (c k) -> p c k", c=CH)
                    nc.vector.tensor_tensor(pl, nsv(2), nsv(nb_off), alu.mult)

                # q = max(PC - (ssc^2*DT)^2, eps)  (custom DVE)
                nc.vector._custom_dve(OPS["QSUB"], out=QQ[:], in0=PC[:],
                                      in1=DT[:], s0=1e-16, s1=ssc * ssc)
                # rq' = 1/sqrt(q), rna' = 1/sqrt(na2')  (Act)
                nc.scalar.activation(RQ[:], QQ[:], AF.Abs_reciprocal_sqrt)
                nc.scalar.activation(
                    RN[:].rearrange("p (c k) -> p c k", c=CH), nsv(2),
                    AF.Abs_reciprocal_sqrt)
                # gate write: 0.0, ordered after this group's abs_rsqrt ops
                nc.scalar.activation(gates[g][:], RN[:, 0:1], AF.Square,
                                     scale=0.0)
                # tc = DT*rq'  (DVE; 1/64 folded into arctan input scale)
                nc.vector.tensor_tensor(TC[:], DT[:], RQ[:], alu.mult)
                # acc = min((na2'*rna'*(a0/ssc) + b0)^2, C0)  (custom DVE)
                nc.vector._custom_dve(OPS["WMIN0B"], out=ACC[:], in0=nsv(2),
                                      in1=RN[:], s0=a0 * isc, s1=b0, imm2=C0)

            # phase B: arctans read the gate as bias, so they are forced
            # after all abs_rsqrt work -> exactly one switch to the trig table
            with tc.high_priority(offset=-(1 << 20)):
                for g, _ in units:
                    TC, ACC = tcs[g], accs[g]
                    AR = wk.tile([128, 2 * GNB], f32, tag=f"AR{g}")
                    nc.scalar.activation(AR[:], TC[:], AF.Arctan,
                                         scale=ssc * ssc,
                                         bias=gates[g][:, 0:1])
                    # feature 1 from d1 plane; feature 2 from raw2 plane
                    # (raw2 = -dot2, sign folded into the a2 constant)
                    nc.vector._custom_dve(OPS["WMIN"], out=ACC[:],
                                          in0=AR[:, GNB:], in1=ACC[:],
                                          s0=a1, s1=b1, imm2=C1)
                    nc.vector._custom_dve(OPS["WMIN"], out=ACC[:],
                                          in0=AR[:, :GNB], in1=ACC[:],
                                          s0=-a2, s1=b2, imm2=C2)
                    nc.sync.dma_start(
                        bass.AP(out, g * 128 * GNB, [[GNB, 128], [1, GNB]]),
                        ACC[:])
    nc.compile()
    return nc


# --------------------------------------------------------------------- run
def _in_maps(coords):
    """Repack coords into the per-core SBUF slab layout [g][p][c][j(585)]."""
    from numpy.lib.stride_tricks import as_strided

    cf = np.ascontiguousarray(coords, dtype=np.float32).reshape(-1)
    cf = np.concatenate([cf, np.full(16, 1.0, dtype=np.float32)])
    s = cf.itemsize
    in_maps = []
    for i in range(NCORES):
        base = cf[i * CORE_F:]
        v = as_strided(base, shape=(BPC, 128, CH, CW),
                       strides=(GRP_F * s, 576 * s, CHAIN_F * s, s))
        in_maps.append({"sarr": np.ascontiguousarray(v).reshape(-1)})
    return in_maps


def _unshard(outs):
    """outs: per-core [BPC*128*CH*K] arrays -> full [B, CH, R, NALT]."""
    full = np.zeros((B, CH, R, NALT), dtype=np.float32)
    for i, o in enumerate(outs):
        o = o.reshape(BPC, 128, CH, K)
        for g in range(BPC):
            v = o[g].transpose(1, 0, 2).reshape(CH, R)  # [c][r=64p+k]
            full[2 * i + g, :, : R - 1, 0] = v[:, : R - 1]
    return full


def _run_fast(coords, consts):
    from concourse.bass_utils import run_bass_kernel_spmd

    if consts not in _BUILT:
        _BUILT[consts] = _build(consts)
    nc = _BUILT[consts]
    res = run_bass_kernel_spmd(nc, _in_maps(coords), core_ids=list(range(NCORES)))
    return _unshard([r["out"] for r in res.results])


def _reference_numpy(atom_description, coords, alternatives, weight, mean, std):
    """Pure-numpy mirror of the jax reference (general-input fallback)."""
    ad = np.asarray(atom_description)
    coords = np.asarray(coords, dtype=np.float32)
    at, resnum, chain, batch, resname = (ad[:, i] for i in range(5))
    n = coords.shape[0]
    table = np.full((B, CH, R, 3), -1, dtype=np.int32)
    table[batch, chain, resnum, at] = np.arange(n, dtype=np.int32)

    c_idx = table[:, :, :-1, 2].reshape(-1)
    n_idx = table[:, :, 1:, 0].reshape(-1)
    cac_idx = table[:, :, :-1, 1].reshape(-1)
    can_idx = table[:, :, 1:, 1].reshape(-1)
    valid = (c_idx >= 0) & (n_idx >= 0) & (cac_idx >= 0) & (can_idx >= 0)

    safe = lambda i: np.where(i >= 0, i, 0)
    cc = coords[safe(c_idx)]
    ncrd = coords[safe(n_idx)]
    cacc = coords[safe(cac_idx)]
    canc = coords[safe(can_idx)]

    def angle_deg(a, b):
        na = np.linalg.norm(a, axis=-1).astype(np.float32)
        nb = np.linalg.norm(b, axis=-1).astype(np.float32)
        mask = (na > 0) & (nb > 0)
        cosang = np.sum(a * b, axis=-1) / np.maximum(na * nb, np.float32(1e-12))
        ang = np.degrees(np.arccos(np.clip(cosang, -1.0, 1.0))).astype(np.float32)
        return ang, mask

    blen = np.linalg.norm(cc - ncrd, axis=-1).astype(np.float32)
    v_cn = ncrd - cc
    ang1, m1 = angle_deg(v_cn, canc - ncrd)
    ang2, m2 = angle_deg(cc - cacc, -v_cn)
    valid = valid & m1 & m2

    x = np.stack([blen, ang1, ang2], axis=-1)
    seq = resname[safe(c_idx)]
    mu = np.asarray(mean, np.float32)[seq]
    var = np.asarray(std, np.float32)[seq] ** 2
    denom = np.sqrt(2.0 * np.pi * var).astype(np.float32)
    pdf = np.exp(-((x - mu) ** 2) / (2.0 * var)) / denom
    score = -(np.log(np.maximum(pdf, np.float32(EPS))) + np.log(denom))
    total = score.sum(-1) * (1.0 - np.tanh(-np.asarray(weight, np.float32)[0]))
    total = np.where(valid, total, np.float32(0.0)).astype(np.float32)

    resi = np.zeros((B, CH, R, NALT), dtype=np.float32)
    resi[:, :, : R - 1, 0] = total.reshape(B, CH, R - 1)
    return resi


def kernel(atom_description, coords, alternatives, weight, mean, std):
    if _check_structured(atom_description, coords, mean, std, weight):
        consts = _consts(mean, std, weight)
        if consts is not None:
            return _run_fast(coords, consts)
    return _reference_numpy(atom_description, coords, alternatives, weight, mean, std)
